# revision 14
# baseline (speedup 1.0000x reference)
"""Trainium2 Bass kernel for nn_DCAM (dense transformer attention module).

Reference computation (per batch b):
  qp/kp/vp = avg_pool2d(feature_{q,k,v}, 2)            # (C=256, 64, 64)
  q = Wq @ qp, k = Wk @ kp  (M=32 channels)            # (32, N=4096)
  v = Wv @ vp                                          # (256, N)
  attn = softmax(q^T k, axis=-1)                       # (N, N)
  out[c, m] = sum_n v[c, n] attn[m, n]                 # (256, N)
  result = upsample_nearest(out, 2) + feature_v        # (256, 128, 128)

Sharding: data-parallel over batch B=8 across 8 NeuronCores (1 batch/core).

Per-core design (v2 — restructured for ACT-bound steady state):
  - Inputs are converted to bf16 on the host: halves input DMA (24 MiB
    vs 48) with error well inside the 2e-2 gate (measured 3.7e-3 in a
    bit-accurate numpy sim of the full pipeline).
  - Single-precision bf16 S matmuls (no hi/lo split): 4 j-blocks packed
    via tile_position row groups -> one 4-way concurrent wave per jg.
  - Phase structure: fv+fk stream in first; k/vt projections run as
    chunks arrive. fq streams PER I-CHUNK underneath phase B, so
    attention starts ~35us in instead of waiting for all input DMA.
  - Phase B emission is software-pipelined: S matmuls for (jg+1) are
    emitted BEFORE the out-matmuls of jg, so the PE queue never head-
    blocks on exp results and ACT (the critical engine at ~2.1us/jg)
    stays saturated. PSUM: s-tiles bufs=2 (4 banks) + o bufs=2 x2 (4).
  - softmax denominator: p tiles accumulated elementwise (DVE/GpSimd
    split), column-summed by 4 ones-matmuls into PSUM per i-chunk;
    reciprocal computed directly on the (1,512) row, broadcast via one
    DRAM bounce.
  - ACT does ONLY exp in phase B (evictions go to DVE/GpSimd): exp is
    the hard floor at 1 elem/cycle/lane -> ~133us/core.
  - pooling is a 2x2 *sum*; scales fold into the exp scale (1/16) and
    into WvT (x0.25) on the host.
"""
import numpy as np
import ml_dtypes

import concourse.bass as bass
import concourse.mybir as mybir
import concourse.tile as tile
from concourse import bacc
from concourse.bass_utils import run_bass_kernel_spmd

F32 = mybir.dt.float32
F32R = mybir.dt.float32r
BF16 = mybir.dt.bfloat16
AF = mybir.ActivationFunctionType

B = 8
C = 256
M = 32
H = W = 128
HP = WP = 64
N = HP * WP          # 4096
CB = C // 128        # 2 channel blocks
JB = N // 128        # 32 key blocks
JG = JB // 4         # 8 groups of 4 packed j-blocks
IC = N // 512        # 8 query chunks


def build_module():
    nc = bacc.Bacc("TRN2", target_bir_lowering=False, debug=False)

    fq_d = nc.dram_tensor("feature_q", [C, H, W], BF16, kind="ExternalInput").ap()
    fk_d = nc.dram_tensor("feature_k", [C, H, W], BF16, kind="ExternalInput").ap()
    fv_d = nc.dram_tensor("feature_v", [C, H, W], BF16, kind="ExternalInput").ap()
    wq_d = nc.dram_tensor("WqT", [C, M], BF16, kind="ExternalInput").ap()
    wk_d = nc.dram_tensor("WkT", [C, M], BF16, kind="ExternalInput").ap()
    wv_d = nc.dram_tensor("WvT", [C, C], BF16, kind="ExternalInput").ap()
    out_d = nc.dram_tensor("out", [C, H, W], F32, kind="ExternalOutput").ap()

    with tile.TileContext(nc) as tc:
        with tc.tile_pool(name="const", bufs=1) as cpool, \
             tc.tile_pool(name="persist", bufs=1) as pp, \
             tc.tile_pool(name="ps", bufs=1, space="PSUM") as ps, \
             tc.tile_pool(name="dramb", bufs=2, space="DRAM") as dpool, \
             tc.tile_pool(name="wk", bufs=1) as wkp:
            # ---- constants ----
            wq_sb = cpool.tile([128, CB, M], BF16, name="wq_sb")
            nc.sync.dma_start(wq_sb[:], wq_d.rearrange("(b p) m -> p b m", p=128))
            wk_sb = cpool.tile([128, CB, M], BF16, name="wk_sb")
            nc.sync.dma_start(wk_sb[:], wk_d.rearrange("(b p) m -> p b m", p=128))
            wv_sb = cpool.tile([128, CB, C], BF16, name="wv_sb")
            nc.sync.dma_start(wv_sb[:], wv_d.rearrange("(b p) c -> p b c", p=128))
            ones_f32r = cpool.tile([128, 1], F32R, name="ones_f32r")
            nc.vector.memset(ones_f32r.bitcast(F32), 1.0)

            # ---- persistent tensors ----
            q4 = pp.tile([128, N], BF16, name="q4")          # q replicated x4
            kh = pp.tile([128, JG, 128], BF16, name="kh")    # [32*t+m, jg, jf]
            vt = pp.tile([128, JB, C], BF16, name="vt")      # vT[j, c] per jb
            fv_sb = pp.tile([128, CB, H, W], BF16, name="fv_sb")

            # fv load first (own queue): hh-major so half-0 pooling can
            # start after 2 of the 4 chunks.
            for hh in range(2):
                for cb in range(CB):
                    nc.gpsimd.dma_start(
                        fv_sb[:, cb, hh * 64:(hh + 1) * 64, :],
                        fv_d[cb * 128:(cb + 1) * 128,
                             hh * 64:(hh + 1) * 64, :])

            # ---- fk: stream chunks, pool, project, pack ----
            for icn in range(IC):
                kp = wkp.tile([128, CB, 8, WP], BF16, tag="kp", bufs=2,
                              name="kp")
                for cb in range(CB):
                    x5 = wkp.tile([128, 8, 2, WP, 2], BF16, tag="xk", bufs=2,
                                  name="x5k")
                    nc.sync.dma_start(
                        x5[:],
                        fk_d[cb * 128:(cb + 1) * 128,
                             icn * 16:(icn + 1) * 16, :].rearrange(
                            "c (h dy) (w dx) -> c h dy w dx", dy=2, dx=2))
                    r = wkp.tile([128, 8, WP, 2], BF16, tag="rk", bufs=2,
                                 name="rk")
                    eng1 = nc.gpsimd if icn % 2 else nc.vector
                    eng1.tensor_add(r[:], x5[:, :, 0], x5[:, :, 1])
                    nc.vector.tensor_add(kp[:, cb], r[:, :, :, 0],
                                         r[:, :, :, 1])
                pr = ps.tile([32, 512], F32, tag="pr", bufs=1,
                             name="prk")
                nc.tensor.matmul(pr[:], wk_sb[:, 0], kp[:, 0], start=True,
                                 stop=False, skip_group_check=True)
                nc.tensor.matmul(pr[:], wk_sb[:, 1], kp[:, 1], start=False,
                                 stop=True, skip_group_check=True)
                for t in range(4):
                    nc.scalar.copy(kh[t * 32:(t + 1) * 32, icn, :],
                                   pr[:, t * 128:(t + 1) * 128])

            # ---- fv: pool halves, project vT per j-block ----
            for half in range(2):
                vph = wkp.tile([128, CB, 32, WP], BF16, tag="vph", bufs=2,
                               name="vph")
                for cb in range(CB):
                    for sub in range(2):
                        raw0 = half * 64 + sub * 32
                        src = fv_sb[:, cb, raw0:raw0 + 32, :].rearrange(
                            "c (h dy) (w dx) -> c h dy w dx", dy=2, dx=2)
                        rfv = wkp.tile([128, 16, WP, 2], BF16, tag="rfv",
                                       bufs=2, name="rfv")
                        eng1 = nc.vector if (cb + sub) % 2 else nc.gpsimd
                        eng1.tensor_add(rfv[:], src[:, :, 0],
                                        src[:, :, 1])
                        nc.gpsimd.tensor_add(
                            vph[:, cb, sub * 16:(sub + 1) * 16, :],
                            rfv[:, :, :, 0], rfv[:, :, :, 1])
                for r2 in range(16):
                    jb = half * 16 + r2
                    vt_ps = ps.tile([128, 512], F32, tag=f"o{r2 % 2}",
                                    bufs=1, name="vt_ps")[:, :C]
                    nc.tensor.matmul(vt_ps, vph[:, 0, r2 * 2:r2 * 2 + 2, :],
                                     wv_sb[:, 0], start=True, stop=False,
                                     skip_group_check=True)
                    nc.tensor.matmul(vt_ps, vph[:, 1, r2 * 2:r2 * 2 + 2, :],
                                     wv_sb[:, 1], start=False, stop=True,
                                     skip_group_check=True)
                    nc.scalar.copy(vt[:, jb, :], vt_ps[:])

            # ---- fq chunk pipeline (ic=0 now; ic+1 inside B) ----
            def q_pipeline(icn, gps_stage1):
                qp = wkp.tile([128, CB, 8, WP], BF16, tag="qp", bufs=2,
                              name="qp")
                for cb in range(CB):
                    x5 = wkp.tile([128, 8, 2, WP, 2], BF16, tag="xq", bufs=2,
                                  name="x5q")
                    nc.sync.dma_start(
                        x5[:],
                        fq_d[cb * 128:(cb + 1) * 128,
                             icn * 16:(icn + 1) * 16, :].rearrange(
                            "c (h dy) (w dx) -> c h dy w dx", dy=2, dx=2))
                    r = wkp.tile([128, 8, WP, 2], BF16, tag="rq", bufs=2,
                                 name="rq")
                    eng = nc.gpsimd if gps_stage1 else nc.vector
                    eng.tensor_add(r[:], x5[:, :, 0], x5[:, :, 1])
                    nc.vector.tensor_add(qp[:, cb], r[:, :, :, 0],
                                         r[:, :, :, 1])
                pr = ps.tile([32, 512], F32, tag="pr", bufs=1,
                             name="prq")
                nc.tensor.matmul(pr[:], wq_sb[:, 0], qp[:, 0], start=True,
                                 stop=False, skip_group_check=True)
                nc.tensor.matmul(pr[:], wq_sb[:, 1], qp[:, 1], start=False,
                                 stop=True, skip_group_check=True)
                cs = slice(icn * 512, (icn + 1) * 512)
                nc.vector.tensor_copy(q4[0:32, cs], pr[:])
                for g in range(1, 4):
                    nc.sync.dma_start(q4[g * 32:(g + 1) * 32, cs],
                                      q4[0:32, cs])

            q_pipeline(0, gps_stage1=False)

            # =========== Phase B: attention + fused epilogue ===========
            # 16 steps per i-chunk, one (128,1024) s-tile per step holding
            # TWO j-blocks (2*st, 2*st+1). Row groups (2*(st%2), 2*(st%2)+1)
            # match the kh packing, so alternating steps use disjoint PE
            # row groups and their S waves overlap.
            NSTEP = 16

            def emit_S(ic, st, s_tile):
                i0 = ic * 512
                jgrp = st // 2
                for u in range(2):
                    t = 2 * (st % 2) + u
                    gs = slice(t * 32, (t + 1) * 32)
                    dst = s_tile[:, u * 512:(u + 1) * 512]
                    nc.tensor.matmul(dst, kh[gs, jgrp, :], q4[gs, i0:i0 + 512],
                                     start=True, stop=True,
                                     tile_position=(t * 32, 0),
                                     skip_group_check=True)

            with tc.tile_pool(name="poolB", bufs=1) as pb:
                def new_s():
                    return ps.tile([128, 1024], F32, tag="s", bufs=2,
                                   name="s_t")

                s_q = [new_s(), new_s()]
                emit_S(0, 0, s_q[0])
                emit_S(0, 1, s_q[1])
                GPS_STEPS = {0, 3, 6, 9, 12}
                for ic in range(IC):
                    o_ps = [ps.tile([128, 512], F32, tag=f"o{cb}", bufs=1,
                                    name=f"o{cb}_ps") for cb in range(CB)]
                    l_ps = ps.tile([1, 512], F32, tag="l", bufs=1,
                                   name="l_ps")
                    lacc_d = pb.tile([128, 1024], F32R, tag="lacc_d", bufs=2,
                                     name="lacc_d")
                    lacc_g = pb.tile([128, 1024], F32R, tag="lacc_g", bufs=2,
                                     name="lacc_g")
                    nc.vector.memset(lacc_d.bitcast(F32), 0.0)
                    nc.gpsimd.memset(lacc_g.bitcast(F32), 0.0)
                    for st in range(NSTEP):
                        s_cur = s_q.pop(0)
                        nxt = (ic, st + 2) if st + 2 < NSTEP else (
                            (ic + 1, st - 14) if ic + 1 < IC else None)
                        if nxt is not None:
                            s_new = new_s()
                            emit_S(nxt[0], nxt[1], s_new)
                            s_q.append(s_new)
                        p = pb.tile([128, 1024], BF16, tag="p", bufs=4,
                                    name="p")
                        nc.scalar.activation(p[:], s_cur[:], AF.Exp,
                                             scale=0.0625)
                        if st in GPS_STEPS:
                            nc.gpsimd.tensor_add(lacc_g[:], lacc_g[:], p[:])
                        else:
                            nc.vector.tensor_add(lacc_d[:], lacc_d[:], p[:])
                        for u in range(2):
                            j = 2 * st + u
                            pr8 = p[:, u * 512:u * 512 + 512]
                            for cb in range(CB):
                                nc.tensor.matmul(
                                    o_ps[cb][:],
                                    vt[:, j, cb * 128:(cb + 1) * 128],
                                    pr8,
                                    start=(j == 0), stop=(j == JB - 1),
                                    skip_group_check=True)
                        if st == 2 and ic + 1 < IC:
                            q_pipeline(ic + 1, gps_stage1=True)
                    # ---- fused epilogue for this i-chunk ----
                    # l = colsum of both accumulators (PSUM-merged by four
                    # ones-matmuls; the two column halves cover the same
                    # i-range, so they accumulate into one (1,512) row)
                    halves = [lacc_d[:, :512], lacc_d[:, 512:],
                              lacc_g[:, :512], lacc_g[:, 512:]]
                    for hi_, hv in enumerate(halves):
                        nc.tensor.matmul(l_ps[:], ones_f32r[:], hv,
                                         start=(hi_ == 0),
                                         stop=(hi_ == len(halves) - 1),
                                         skip_group_check=True)
                    # free the o banks fast: evict raw accumulators to SBUF
                    # on DVE, normalize from there once 1/l arrives.
                    o_sb = []
                    for cb in range(CB):
                        t_ = pb.tile([128, 512], F32, tag=f"osb{cb}", bufs=2,
                                     name=f"osb{cb}")
                        nc.vector.tensor_copy(t_[:], o_ps[cb][:])
                        o_sb.append(t_)
                    # l -> 1/l broadcast via DRAM bounce (transpose to
                    # (128,4) so the reciprocal runs wide on DVE lanes)
                    l_sb = pb.tile([1, 512], F32, tag="l_sb", bufs=2,
                                   name="l_sb")
                    nc.scalar.copy(l_sb[:], l_ps[:])
                    l_dr = dpool.tile([512], F32, tag="l_dr", bufs=2,
                                      name="l_dr")
                    nc.sync.dma_start(l_dr[:], l_sb[:])
                    lT = pb.tile([128, 4], F32, tag="lT", bufs=2, name="lT")
                    nc.sync.dma_start(lT[:], l_dr.rearrange("(p b) -> p b",
                                                            b=4))
                    rT = pb.tile([128, 4], F32, tag="rT", bufs=2, name="rT")
                    nc.vector.reciprocal(rT[:], lT[:])
                    r_dr = dpool.tile([512], F32, tag="r_dr", bufs=2,
                                      name="r_dr")
                    nc.sync.dma_start(r_dr.rearrange("(p b) -> p b", b=4),
                                      rT[:])
                    rb = pb.tile([128, 512], F32, tag="rb", bufs=2,
                                 name="rb")
                    nc.sync.dma_start(
                        rb[:],
                        r_dr.rearrange("(o x) -> o x", o=1).to_broadcast(
                            (128, 512)))
                    for cb in range(CB):
                        oc = pb.tile([128, 512], BF16, tag="oc", bufs=4,
                                     name="oc")
                        nc.vector.tensor_mul(oc[:], o_sb[cb][:], rb[:])
                        final = pb.tile([128, 8, 2, WP, 2], F32, tag="final",
                                        bufs=2, name="final")
                        up = oc.rearrange("c (h w) -> c h w", w=WP)[
                            :, :, :, None].to_broadcast((128, 8, WP, 2))
                        fvv = fv_sb[:, cb,
                                    ic * 16:(ic + 1) * 16, :].rearrange(
                            "c (h dy) (w dx) -> c h dy w dx", dy=2, dx=2)
                        nc.vector.tensor_add(final[:, :, 0], up,
                                             fvv[:, :, 0])
                        nc.gpsimd.tensor_add(final[:, :, 1], up,
                                             fvv[:, :, 1])
                        nc.sync.dma_start(
                            out_d[cb * 128:(cb + 1) * 128,
                                  ic * 16:(ic + 1) * 16, :],
                            final.rearrange("c h dy w dx -> c (h dy) (w dx)"))

    nc.compile()
    return nc


_NC_CACHE = []
LAST_RESULT = []  # last BassKernelResults, for perf inspection by test.py


def kernel(**inputs) -> np.ndarray:
    bf = ml_dtypes.bfloat16
    fq = np.ascontiguousarray(np.asarray(inputs["feature_q"]).astype(bf))
    fk = np.ascontiguousarray(np.asarray(inputs["feature_k"]).astype(bf))
    fv = np.ascontiguousarray(np.asarray(inputs["feature_v"]).astype(bf))
    wq = np.asarray(inputs["Wq"], dtype=np.float32)
    wk = np.asarray(inputs["Wk"], dtype=np.float32)
    wv = np.asarray(inputs["Wv"], dtype=np.float32)

    # weight layout prep: on-device pooling is a 2x2 *sum*; q,k each pick
    # up 4x -> s is 16x, folded into the on-device exp scale; v's 4x is
    # folded into WvT here.
    wqt = np.ascontiguousarray(wq.T.astype(bf))               # (C, M)
    wkt = np.ascontiguousarray(wk.T.astype(bf))
    wvt = np.ascontiguousarray((wv.T * 0.25).astype(bf))      # (C, C)

    if not _NC_CACHE:
        _NC_CACHE.append(build_module())
    nc = _NC_CACHE[0]

    in_maps = [
        {
            "feature_q": fq[b],
            "feature_k": fk[b],
            "feature_v": fv[b],
            "WqT": wqt,
            "WkT": wkt,
            "WvT": wvt,
        }
        for b in range(B)
    ]
    res = run_bass_kernel_spmd(nc, in_maps, core_ids=list(range(B)))
    LAST_RESULT.clear()
    LAST_RESULT.append(res)
    out = np.stack([res.results[b]["out"] for b in range(B)], axis=0)
    return out.astype(np.float32)


if __name__ == "__main__":
    nc = build_module()
    print("module built + compiled OK")


# revision 19
# speedup vs baseline: 1.1169x; 1.1169x over previous
"""Trainium2 Bass kernel for nn_DCAM (dense transformer attention module).

Reference computation (per batch b):
  qp/kp/vp = avg_pool2d(feature_{q,k,v}, 2)            # (C=256, 64, 64)
  q = Wq @ qp, k = Wk @ kp  (M=32 channels)            # (32, N=4096)
  v = Wv @ vp                                          # (256, N)
  attn = softmax(q^T k, axis=-1)                       # (N, N)
  out[c, m] = sum_n v[c, n] attn[m, n]                 # (256, N)
  result = upsample_nearest(out, 2) + feature_v        # (256, 128, 128)

Sharding: data-parallel over batch B=8 across 8 NeuronCores (1 batch/core).

Per-core design (v2 — restructured for ACT-bound steady state):
  - Inputs are converted to bf16 on the host: halves input DMA (24 MiB
    vs 48) with error well inside the 2e-2 gate (measured 3.7e-3 in a
    bit-accurate numpy sim of the full pipeline).
  - Single-precision bf16 S matmuls (no hi/lo split): 4 j-blocks packed
    via tile_position row groups -> one 4-way concurrent wave per jg.
  - Phase structure: fv+fk stream in first; k/vt projections run as
    chunks arrive. fq streams PER I-CHUNK underneath phase B, so
    attention starts ~35us in instead of waiting for all input DMA.
  - Phase B emission is software-pipelined: S matmuls for (jg+1) are
    emitted BEFORE the out-matmuls of jg, so the PE queue never head-
    blocks on exp results and ACT (the critical engine at ~2.1us/jg)
    stays saturated. PSUM: s-tiles bufs=2 (4 banks) + o bufs=2 x2 (4).
  - softmax denominator: p tiles accumulated elementwise (DVE/GpSimd
    split), column-summed by 4 ones-matmuls into PSUM per i-chunk;
    reciprocal computed directly on the (1,512) row, broadcast via one
    DRAM bounce.
  - ACT does ONLY exp in phase B (evictions go to DVE/GpSimd): exp is
    the hard floor at 1 elem/cycle/lane -> ~133us/core.
  - pooling is a 2x2 *sum*; scales fold into the exp scale (1/16) and
    into WvT (x0.25) on the host.
"""
import numpy as np
import ml_dtypes

import concourse.bass as bass
import concourse.mybir as mybir
import concourse.tile as tile
from concourse import bacc
from concourse.bass_utils import run_bass_kernel_spmd

F32 = mybir.dt.float32
F32R = mybir.dt.float32r
BF16 = mybir.dt.bfloat16
AF = mybir.ActivationFunctionType

B = 8
C = 256
M = 32
H = W = 128
HP = WP = 64
N = HP * WP          # 4096
CB = C // 128        # 2 channel blocks
JB = N // 128        # 32 key blocks
JG = JB // 4         # 8 groups of 4 packed j-blocks
IC = N // 512        # 8 query chunks


def build_module():
    nc = bacc.Bacc("TRN2", target_bir_lowering=False, debug=False)

    fq_d = nc.dram_tensor("feature_q", [C, H, W], BF16, kind="ExternalInput").ap()
    fk_d = nc.dram_tensor("feature_k", [C, H, W], BF16, kind="ExternalInput").ap()
    fv_d = nc.dram_tensor("feature_v", [C, H, W], BF16, kind="ExternalInput").ap()
    wq_d = nc.dram_tensor("WqT", [C, M], BF16, kind="ExternalInput").ap()
    wk_d = nc.dram_tensor("WkT", [C, M], BF16, kind="ExternalInput").ap()
    wv_d = nc.dram_tensor("WvT", [C, C], BF16, kind="ExternalInput").ap()
    out_d = nc.dram_tensor("out", [C, H, W], F32, kind="ExternalOutput").ap()

    with tile.TileContext(nc) as tc:
        with tc.tile_pool(name="const", bufs=1) as cpool, \
             tc.tile_pool(name="persist", bufs=1) as pp, \
             tc.tile_pool(name="ps", bufs=1, space="PSUM") as ps, \
             tc.tile_pool(name="dramb", bufs=2, space="DRAM") as dpool, \
             tc.tile_pool(name="wk", bufs=1) as wkp:
            # ---- constants ----
            wq_sb = cpool.tile([128, CB, M], BF16, name="wq_sb")
            nc.sync.dma_start(wq_sb[:], wq_d.rearrange("(b p) m -> p b m", p=128))
            wk_sb = cpool.tile([128, CB, M], BF16, name="wk_sb")
            nc.sync.dma_start(wk_sb[:], wk_d.rearrange("(b p) m -> p b m", p=128))
            wv_sb = cpool.tile([128, CB, C], BF16, name="wv_sb")
            nc.sync.dma_start(wv_sb[:], wv_d.rearrange("(b p) c -> p b c", p=128))
            ones_col = cpool.tile([128, 1], BF16, name="ones_col")
            nc.vector.memset(ones_col, 1.0)

            # ---- persistent tensors ----
            q4 = pp.tile([128, N], BF16, name="q4")          # q replicated x4
            kh = pp.tile([128, JG, 128], BF16, name="kh")    # [32*t+m, jg, jf]
            vt = pp.tile([128, JB, C], BF16, name="vt")      # vT[j, c] per jb
            fv_sb = pp.tile([128, CB, H, W], BF16, name="fv_sb")

            # fv load first (own queue): hh-major so half-0 pooling can
            # start after 2 of the 4 chunks.
            for hh in range(2):
                for cb in range(CB):
                    nc.gpsimd.dma_start(
                        fv_sb[:, cb, hh * 64:(hh + 1) * 64, :],
                        fv_d[cb * 128:(cb + 1) * 128,
                             hh * 64:(hh + 1) * 64, :])

            # ---- fk: stream chunks, pool, project, pack ----
            for icn in range(IC):
                kp = wkp.tile([128, CB, 8, WP], BF16, tag="kp", bufs=2,
                              name="kp")
                for cb in range(CB):
                    x5 = wkp.tile([128, 8, 2, WP, 2], BF16, tag="xk", bufs=2,
                                  name="x5k")
                    nc.sync.dma_start(
                        x5[:],
                        fk_d[cb * 128:(cb + 1) * 128,
                             icn * 16:(icn + 1) * 16, :].rearrange(
                            "c (h dy) (w dx) -> c h dy w dx", dy=2, dx=2))
                    r = wkp.tile([128, 8, WP, 2], BF16, tag="rk", bufs=2,
                                 name="rk")
                    nc.vector.tensor_add(r[:], x5[:, :, 0], x5[:, :, 1])
                    nc.vector.tensor_add(kp[:, cb], r[:, :, :, 0],
                                         r[:, :, :, 1])
                pr = ps.tile([32, 512], F32, tag="pr", bufs=1,
                             name="prk")
                nc.tensor.matmul(pr[:], wk_sb[:, 0], kp[:, 0], start=True,
                                 stop=False, skip_group_check=True)
                nc.tensor.matmul(pr[:], wk_sb[:, 1], kp[:, 1], start=False,
                                 stop=True, skip_group_check=True)
                for t in range(4):
                    nc.scalar.copy(kh[t * 32:(t + 1) * 32, icn, :],
                                   pr[:, t * 128:(t + 1) * 128])

            # ---- fv: pool halves, project vT per j-block ----
            for half in range(2):
                vph = wkp.tile([128, CB, 32, WP], BF16, tag="vph", bufs=2,
                               name="vph")
                for cb in range(CB):
                    for sub in range(2):
                        raw0 = half * 64 + sub * 32
                        src = fv_sb[:, cb, raw0:raw0 + 32, :].rearrange(
                            "c (h dy) (w dx) -> c h dy w dx", dy=2, dx=2)
                        rfv = wkp.tile([128, 16, WP, 2], BF16, tag="rfv",
                                       bufs=2, name="rfv")
                        nc.gpsimd.tensor_add(rfv[:], src[:, :, 0],
                                             src[:, :, 1])
                        nc.gpsimd.tensor_add(
                            vph[:, cb, sub * 16:(sub + 1) * 16, :],
                            rfv[:, :, :, 0], rfv[:, :, :, 1])
                for r2 in range(16):
                    jb = half * 16 + r2
                    vt_ps = ps.tile([128, 512], F32, tag=f"o{r2 % 2}",
                                    bufs=1, name="vt_ps")[:, :C]
                    nc.tensor.matmul(vt_ps, vph[:, 0, r2 * 2:r2 * 2 + 2, :],
                                     wv_sb[:, 0], start=True, stop=False,
                                     skip_group_check=True)
                    nc.tensor.matmul(vt_ps, vph[:, 1, r2 * 2:r2 * 2 + 2, :],
                                     wv_sb[:, 1], start=False, stop=True,
                                     skip_group_check=True)
                    nc.scalar.copy(vt[:, jb, :], vt_ps[:])

            # ---- fq chunk pipeline (ic=0 now; ic+1 inside B) ----
            def q_pipeline(icn, gps_stage1):
                qp = wkp.tile([128, CB, 8, WP], BF16, tag="qp", bufs=2,
                              name="qp")
                for cb in range(CB):
                    x5 = wkp.tile([128, 8, 2, WP, 2], BF16, tag="xq", bufs=2,
                                  name="x5q")
                    nc.sync.dma_start(
                        x5[:],
                        fq_d[cb * 128:(cb + 1) * 128,
                             icn * 16:(icn + 1) * 16, :].rearrange(
                            "c (h dy) (w dx) -> c h dy w dx", dy=2, dx=2))
                    r = wkp.tile([128, 8, WP, 2], BF16, tag="rq", bufs=2,
                                 name="rq")
                    eng = nc.gpsimd if gps_stage1 else nc.vector
                    eng.tensor_add(r[:], x5[:, :, 0], x5[:, :, 1])
                    nc.vector.tensor_add(qp[:, cb], r[:, :, :, 0],
                                         r[:, :, :, 1])
                pr = ps.tile([32, 512], F32, tag="pr", bufs=1,
                             name="prq")
                nc.tensor.matmul(pr[:], wq_sb[:, 0], qp[:, 0], start=True,
                                 stop=False, skip_group_check=True)
                nc.tensor.matmul(pr[:], wq_sb[:, 1], qp[:, 1], start=False,
                                 stop=True, skip_group_check=True)
                cs = slice(icn * 512, (icn + 1) * 512)
                nc.vector.tensor_copy(q4[0:32, cs], pr[:])
                for g in range(1, 4):
                    nc.sync.dma_start(q4[g * 32:(g + 1) * 32, cs],
                                      q4[0:32, cs])

            q_pipeline(0, gps_stage1=False)

            # =========== Phase B: attention + fused epilogue ===========
            # 16 steps per i-chunk, one (128,1024) s-tile per step holding
            # TWO j-blocks (2*st, 2*st+1). Row groups (2*(st%2), 2*(st%2)+1)
            # match the kh packing, so alternating steps use disjoint PE
            # row groups and their S waves overlap.
            NSTEP = 16

            def emit_S(ic, st, s_tile):
                i0 = ic * 512
                jgrp = st // 2
                for u in range(2):
                    t = 2 * (st % 2) + u
                    gs = slice(t * 32, (t + 1) * 32)
                    dst = s_tile[:, u * 512:(u + 1) * 512]
                    nc.tensor.matmul(dst, kh[gs, jgrp, :], q4[gs, i0:i0 + 512],
                                     start=True, stop=True,
                                     tile_position=(t * 32, 0),
                                     skip_group_check=True)

            with tc.tile_pool(name="poolB", bufs=1) as pb:
                def new_s():
                    return ps.tile([128, 1024], F32, tag="s", bufs=2,
                                   name="s_t")

                s_q = [new_s(), new_s()]
                emit_S(0, 0, s_q[0])
                emit_S(0, 1, s_q[1])
                for ic in range(IC):
                    o_ps = [ps.tile([128, 512], F32, tag=f"o{cb}", bufs=1,
                                    name=f"o{cb}_ps") for cb in range(CB)]
                    l_ps = ps.tile([1, 512], F32, tag="l", bufs=1,
                                   name="l_ps")
                    for st in range(NSTEP):
                        s_cur = s_q.pop(0)
                        nxt = (ic, st + 2) if st + 2 < NSTEP else (
                            (ic + 1, st - 14) if ic + 1 < IC else None)
                        if nxt is not None:
                            s_new = new_s()
                            emit_S(nxt[0], nxt[1], s_new)
                            s_q.append(s_new)
                        p = pb.tile([128, 1024], BF16, tag="p", bufs=4,
                                    name="p")
                        nc.scalar.activation(p[:], s_cur[:], AF.Exp,
                                             scale=0.0625)
                        # l += colsum(p) for both j-block halves (same i's)
                        for u in range(2):
                            nc.tensor.matmul(
                                l_ps[:], ones_col[:],
                                p[:, u * 512:(u + 1) * 512],
                                start=(st == 0 and u == 0),
                                stop=(st == NSTEP - 1 and u == 1),
                                skip_group_check=True)
                        for u in range(2):
                            j = 2 * st + u
                            pr8 = p[:, u * 512:u * 512 + 512]
                            for cb in range(CB):
                                nc.tensor.matmul(
                                    o_ps[cb][:],
                                    vt[:, j, cb * 128:(cb + 1) * 128],
                                    pr8,
                                    start=(j == 0), stop=(j == JB - 1),
                                    skip_group_check=True)
                        if st == 2 and ic + 1 < IC:
                            q_pipeline(ic + 1, gps_stage1=True)
                    # ---- fused epilogue for this i-chunk ----
                    # free the o banks fast: evict raw accumulators to SBUF
                    # on DVE, normalize from there once 1/l arrives.
                    o_sb = []
                    for cb in range(CB):
                        t_ = pb.tile([128, 512], F32, tag=f"osb{cb}", bufs=2,
                                     name=f"osb{cb}")
                        nc.vector.tensor_copy(t_[:], o_ps[cb][:])
                        o_sb.append(t_)
                    # l -> 1/l broadcast via DRAM bounce (transpose to
                    # (128,4) so the reciprocal runs wide on DVE lanes)
                    l_sb = pb.tile([1, 512], F32, tag="l_sb", bufs=2,
                                   name="l_sb")
                    nc.scalar.copy(l_sb[:], l_ps[:])
                    l_dr = dpool.tile([512], F32, tag="l_dr", bufs=2,
                                      name="l_dr")
                    nc.sync.dma_start(l_dr[:], l_sb[:])
                    lT = pb.tile([128, 4], F32, tag="lT", bufs=2, name="lT")
                    nc.sync.dma_start(lT[:], l_dr.rearrange("(p b) -> p b",
                                                            b=4))
                    rT = pb.tile([128, 4], F32, tag="rT", bufs=2, name="rT")
                    nc.vector.reciprocal(rT[:], lT[:])
                    r_dr = dpool.tile([512], F32, tag="r_dr", bufs=2,
                                      name="r_dr")
                    nc.sync.dma_start(r_dr.rearrange("(p b) -> p b", b=4),
                                      rT[:])
                    rb = pb.tile([128, 512], F32, tag="rb", bufs=2,
                                 name="rb")
                    nc.sync.dma_start(
                        rb[:],
                        r_dr.rearrange("(o x) -> o x", o=1).to_broadcast(
                            (128, 512)))
                    for cb in range(CB):
                        oc = pb.tile([128, 512], BF16, tag="oc", bufs=4,
                                     name="oc")
                        nc.vector.tensor_mul(oc[:], o_sb[cb][:], rb[:])
                        final = pb.tile([128, 8, 2, WP, 2], F32, tag="final",
                                        bufs=2, name="final")
                        up = oc.rearrange("c (h w) -> c h w", w=WP)[
                            :, :, :, None].to_broadcast((128, 8, WP, 2))
                        fvv = fv_sb[:, cb,
                                    ic * 16:(ic + 1) * 16, :].rearrange(
                            "c (h dy) (w dx) -> c h dy w dx", dy=2, dx=2)
                        nc.vector.tensor_add(final[:, :, 0], up,
                                             fvv[:, :, 0])
                        nc.gpsimd.tensor_add(final[:, :, 1], up,
                                             fvv[:, :, 1])
                        nc.sync.dma_start(
                            out_d[cb * 128:(cb + 1) * 128,
                                  ic * 16:(ic + 1) * 16, :],
                            final.rearrange("c h dy w dx -> c (h dy) (w dx)"))

    nc.compile()
    return nc


_NC_CACHE = []
LAST_RESULT = []  # last BassKernelResults, for perf inspection by test.py


def kernel(**inputs) -> np.ndarray:
    bf = ml_dtypes.bfloat16
    fq = np.ascontiguousarray(np.asarray(inputs["feature_q"]).astype(bf))
    fk = np.ascontiguousarray(np.asarray(inputs["feature_k"]).astype(bf))
    fv = np.ascontiguousarray(np.asarray(inputs["feature_v"]).astype(bf))
    wq = np.asarray(inputs["Wq"], dtype=np.float32)
    wk = np.asarray(inputs["Wk"], dtype=np.float32)
    wv = np.asarray(inputs["Wv"], dtype=np.float32)

    # weight layout prep: on-device pooling is a 2x2 *sum*; q,k each pick
    # up 4x -> s is 16x, folded into the on-device exp scale; v's 4x is
    # folded into WvT here.
    wqt = np.ascontiguousarray(wq.T.astype(bf))               # (C, M)
    wkt = np.ascontiguousarray(wk.T.astype(bf))
    wvt = np.ascontiguousarray((wv.T * 0.25).astype(bf))      # (C, C)

    if not _NC_CACHE:
        _NC_CACHE.append(build_module())
    nc = _NC_CACHE[0]

    in_maps = [
        {
            "feature_q": fq[b],
            "feature_k": fk[b],
            "feature_v": fv[b],
            "WqT": wqt,
            "WkT": wkt,
            "WvT": wvt,
        }
        for b in range(B)
    ]
    res = run_bass_kernel_spmd(nc, in_maps, core_ids=list(range(B)))
    LAST_RESULT.clear()
    LAST_RESULT.append(res)
    out = np.stack([res.results[b]["out"] for b in range(B)], axis=0)
    return out.astype(np.float32)


if __name__ == "__main__":
    nc = build_module()
    print("module built + compiled OK")
